# revision 4
# baseline (speedup 1.0000x reference)
"""Trainium2 Bass kernel for nn_MinusSpan — fp16, v17.

v2's proven gather topology (8 indirect DMAs, 128 x 2 KB descriptors,
natural span order) plus trace-driven mechanical fixes:
  * idx DMA is issued BEFORE the Block body (right after the framework
    preamble) so its ~2.3 us latency overlaps block entry;
  * in-place DVE subtracts -> one whole-tile [128, 2048] fp16 store per
    chunk (4 KB descriptors) alternating sync/scalar queues;
  * the LAST chunk's store is split in half across BOTH queues, each half
    gated only on its own subtract, shrinking the dependent tail;
  * idx is a padded [128, 128] int32 tile (512 B descriptors).

Device row layout: [diff_f | b_post | f_pre | diff_b]; host reorders
columns and casts to f32 during reassembly.
"""
import numpy as np
from contextlib import ExitStack

import concourse.bass as bass
from concourse import bacc, mybir
from concourse.bass_utils import run_bass_kernel_spmd

B, T, D = 16, 2048, 1024
H = D // 2
N = 256
NCORES = 8
BPC = B // NCORES
S = 2 * T + 6
NP2 = BPC * S - 3
NSP = BPC * N
NBLK = NSP // 128
IDXW = 128

_NC = None


def _build():
    nc = bacc.Bacc("TRN2", target_bir_lowering=False, debug=False,
                   num_devices=NCORES, monotonic_sem_count=0,
                   detect_race_conditions=False)
    p2 = nc.dram_tensor("p2", [NP2, 2 * H], mybir.dt.float16,
                        kind="ExternalInput")
    idx = nc.dram_tensor("idx", [128, IDXW], mybir.dt.int32,
                         kind="ExternalInput")
    out = nc.dram_tensor("out", [NSP, 4 * H], mybir.dt.float16,
                         kind="ExternalOutput")

    with ExitStack() as ctx:
        en = ctx.enter_context
        idx_t = en(nc.sbuf_tensor("idx_t", [128, IDXW], mybir.dt.int32))
        sem_idx = nc.alloc_semaphore("sem_idx")
        # issue the idx load before the block body so its latency overlaps
        # the block-entry handshake
        nc.sync.dma_start(idx_t[:], idx[:]).then_inc(sem_idx, 16)

        block = en(nc.Block(no_gpsimd_drain=True))
        c = [en(nc.sbuf_tensor(f"c_{k}", [128, 4 * H], mybir.dt.float16))
             for k in range(NBLK)]
        sem_g1 = [en(nc.semaphore(f"sem_g1{k}")) for k in range(NBLK)]
        sem_g2 = [en(nc.semaphore(f"sem_g2{k}")) for k in range(NBLK)]
        sem_s1 = [en(nc.semaphore(f"sem_s1{k}")) for k in range(NBLK)]
        sem_s2 = [en(nc.semaphore(f"sem_s2{k}")) for k in range(NBLK)]
        sem_oa = en(nc.semaphore("sem_oa"))
        sem_ob = en(nc.semaphore("sem_ob"))

        @block.sync
        def _(sync: bass.BassEngine):
            k = 0
            rows = out[k * 128:(k + 1) * 128, :]
            sync.wait_ge(sem_s1[k], 1)
            sync.wait_ge(sem_s2[k], 1)
            sync.dma_start(rows, c[k][:, :]).then_inc(sem_oa, 16)
            k = NBLK - 1                          # last chunk quarters
            rows = out[k * 128:(k + 1) * 128, :]
            sync.wait_ge(sem_g1[k], 16)           # b_post: raw, gather-1 only
            sync.dma_start(rows[:, H:2 * H], c[k][:, H:2 * H])\
                .then_inc(sem_oa, 16)
            sync.wait_ge(sem_s1[k], 1)            # diff_f: after sub1
            sync.dma_start(rows[:, 0:H], c[k][:, 0:H]).then_inc(sem_oa, 16)
            sync.wait_ge(sem_oa, 16 * 3)

        @block.gpsimd
        def _(gpsimd: bass.BassGpSimd):
            gpsimd.wait_ge(sem_idx, 16)
            for k in range(NBLK):
                gpsimd.indirect_dma_start(
                    out=c[k][:, 0:2 * H], out_offset=None, in_=p2[:],
                    in_offset=bass.IndirectOffsetOnAxis(
                        ap=idx_t[:, 2 * k:2 * k + 1], axis=0),
                ).then_inc(sem_g1[k], 16)
                gpsimd.indirect_dma_start(
                    out=c[k][:, 2 * H:4 * H], out_offset=None, in_=p2[:],
                    in_offset=bass.IndirectOffsetOnAxis(
                        ap=idx_t[:, 2 * k + 1:2 * k + 2], axis=0),
                ).then_inc(sem_g2[k], 16)

        @block.vector
        def _(vector: bass.BassEngine):
            for k in range(NBLK):
                vector.wait_ge(sem_g1[k], 16)
                vector.wait_ge(sem_g2[k], 16)
                vector.tensor_tensor(
                    out=c[k][:, 0:H], in0=c[k][:, 0:H],
                    in1=c[k][:, 2 * H:3 * H],
                    op=mybir.AluOpType.subtract).then_inc(sem_s1[k], 1)
                vector.tensor_tensor(
                    out=c[k][:, 3 * H:4 * H], in0=c[k][:, 3 * H:4 * H],
                    in1=c[k][:, H:2 * H],
                    op=mybir.AluOpType.subtract).then_inc(sem_s2[k], 1)

        @block.scalar
        def _(scalar: bass.BassEngine):
            for k in (1, 2):
                rows = out[k * 128:(k + 1) * 128, :]
                scalar.wait_ge(sem_s1[k], 1)
                scalar.wait_ge(sem_s2[k], 1)
                scalar.dma_start(rows, c[k][:, :]).then_inc(sem_ob, 16)
            k = NBLK - 1                          # last chunk quarters
            rows = out[k * 128:(k + 1) * 128, :]
            scalar.wait_ge(sem_g2[k], 16)         # f_pre: raw, gather-2 only
            scalar.dma_start(rows[:, 2 * H:3 * H], c[k][:, 2 * H:3 * H])\
                .then_inc(sem_ob, 16)
            scalar.wait_ge(sem_s2[k], 1)          # diff_b: after sub2
            scalar.dma_start(rows[:, 3 * H:4 * H], c[k][:, 3 * H:4 * H])\
                .then_inc(sem_ob, 16)
            scalar.wait_ge(sem_ob, 16 * 4)

    nc.compile()
    return nc


def _prep_core(input_c: np.ndarray, span_c: np.ndarray) -> dict:
    xs = np.asarray(input_c).astype(np.float16).reshape(BPC, 2 * T, H)
    hrp = np.zeros((BPC * S, H), np.float16)
    for b in range(BPC):
        hrp[b * S + 2:b * S + 2 + 2 * T] = xs[b]
    p2 = np.concatenate([hrp[:-3], hrp[3:]], axis=1)  # [NP2, 1024] fp16

    i = span_c[..., 0].astype(np.int64)
    j = span_c[..., 1].astype(np.int64)
    base = (np.arange(BPC, dtype=np.int64) * S)[:, None]
    e1 = base + 2 + 2 * j
    e2 = base + 2 * i
    skip = (i == 0) & (j == 0)
    zv = base + 2 + 2 * T
    e1 = np.where(skip, zv, e1)
    e2 = np.where(skip, zv, e2)
    kinds = np.stack([e1, e2], axis=-1)
    idx = np.full((128, IDXW), int(zv[0, 0]), np.int32)
    idx[:, :NBLK * 2] = (kinds.reshape(BPC, 2, 128, 2)
                         .transpose(2, 0, 1, 3)
                         .reshape(128, NBLK * 2))
    return {"p2": p2, "idx": idx}


def _run(inputs: dict, trace: bool = False, **kw):
    global _NC
    if _NC is None:
        _NC = _build()
    inp = np.asarray(inputs["input"])
    spans = np.asarray(inputs["span_idxs"])
    in_maps = [
        _prep_core(inp[c * BPC:(c + 1) * BPC], spans[c * BPC:(c + 1) * BPC])
        for c in range(NCORES)
    ]
    res = run_bass_kernel_spmd(_NC, in_maps, core_ids=list(range(NCORES)),
                               trace=trace, **kw)
    parts = []
    for cid in range(NCORES):
        dev = res.results[cid]["out"]
        fc = np.concatenate(
            [dev[:, 0:H], dev[:, 3 * H:4 * H],
             dev[:, 2 * H:3 * H], dev[:, H:2 * H]], axis=1,
        ).astype(np.float32)
        parts.append(fc.reshape(BPC, N, 4 * H))
    return np.concatenate(parts, axis=0), res


def kernel(input: np.ndarray, span_idxs: np.ndarray) -> np.ndarray:
    full, _ = _run({"input": input, "span_idxs": span_idxs})
    return full
